# revision 9
# baseline (speedup 1.0000x reference)
"""Trainium2 Bass kernel for nn_Contraction (MACE-style CG contraction), v3.

Math (per node b, channel c):
  wn3 = w_max[elem_b] (23,C); wn2 = w2[elem_b] (5,C); wn1 = w1[elem_b] (1,C)
  c2[w,x2,v] = sum_ik U3[w,x2,v,i,k] (x[c,i] wn3[k,c]) + sum_k2 U2[w,x2,v,k2] wn2[k2,c]
  out[c, w]  = sum_{x2,v} c2[w,x2,v] x[c,x2] x[c,v] + sum_x2 U1[w,x2] wn1[c] x[c,x2]

Key reduction: the final sum over (x2, v) is a quadratic form in x, so only
the (x2, v)-symmetric part of c2 matters, and c2 is linear in U3/U2. Host
pre-symmetrizes U3/U2 over (x2, v): 136 columns per w (120 pair-sums + 16
diagonal) instead of 256 — halves the main-matmul moving stream.

Device mapping (per core, BS=128 nodes, one node at a time):
  - transposed main matmul: stationary = t4_n [(ik) 374 rows, c=128]
    (host-built: x*wn3 rows; chunk2 rows 112:117 = wn2 for the folded U2
    term, row 117 = wn1 for the folded U1 term), moving = u3cat
    [(ik), 408] (bf16, const in SBUF). Output out1T[c, 408] f32, ONE PSUM
    bank. Columns: [w0 136 | w1 136 | w2 136].
    3 matmuls/node (one per K chunk 128/128/118).
  - nodes are processed in batches of 4 to amortize per-op fixed
    costs: PSUM tile [128, 4, 512] (4 banks x 2 bufs = all 8 banks; the
    end-phase pool opens after this one closes), ACT convert-copies all
    4 nodes' 408 cols to SBUF bf16 in one op.
  - final contraction: multiply o1 by xx_n[c, 136] broadcast over w
    (DVE takes w=0; GPSIMD takes w=1,2), then one DVE windowed
    tensor_reduce(axis=X) [c,12,136]->[c,12] writes straight into
    outsb[c, (b, w)] f32. The U1 terms (host-computed u1c) are added
    once at the end in a single batched add.
  - end phase: 3 PE transposes [C,BS] -> [BS,C] into (b, c, w) layout,
    single contiguous DMA out.

Sharding: data-parallel over nodes b across 8 cores (128 nodes/core).
Host prep (numpy): elem gather, t4/u3cat/xx/u1c packing, bf16.
"""

import sys

if "/opt/trn_rl_repo" not in sys.path:
    sys.path.insert(0, "/opt/trn_rl_repo")

import numpy as np
import ml_dtypes

import concourse.bass as bass
import concourse.mybir as mybir
import concourse.tile as tile
from concourse.masks import make_identity

dt = mybir.dt
bf16 = ml_dtypes.bfloat16

# problem constants (hardcoded per contract)
B, C, ELL, EQ, E = 1024, 128, 16, 3, 10
P3, P2, P1 = 23, 5, 1
N_CORES = 8
BS = B // N_CORES          # nodes per core
NB = 4                     # nodes per DMA batch (2 pairs)
NPAIR = ELL * (ELL - 1) // 2      # 120 off-diagonal (x2<v) pairs
WCOL = NPAIR + ELL                # 136 columns per w
NCOL = EQ * WCOL                  # 408 total out1 columns
KCH = (128, 128, 118)      # K chunks (chunk2: 112 U3 + 5 U2 + 1 U1 rows)

_f32 = dt.float32
_bf = dt.bfloat16


def _build_program():
    nc = bass.Bass("TRN2", target_bir_lowering=False, debug=False)

    trep_d = nc.dram_tensor("trep", [128, 3, BS, C], _bf, kind="ExternalInput")
    xx_d = nc.dram_tensor("xx", [C, BS, WCOL], _bf, kind="ExternalInput")
    u3_d = nc.dram_tensor("u3cat", [3, 128, NCOL], _bf, kind="ExternalInput")
    u1c_d = nc.dram_tensor("u1c", [C, BS, EQ], _f32, kind="ExternalInput")
    out_d = nc.dram_tensor("out", [BS, C * EQ], _f32, kind="ExternalOutput")

    mult = mybir.AluOpType.mult
    add = mybir.AluOpType.add

    with tile.TileContext(nc) as tc:
        with tc.tile_pool(name="const", bufs=1) as cpool:
            u3sb = cpool.tile([128, 3, NCOL], _bf)
            nc.sync.dma_start(out=u3sb[:], in_=u3_d[:].rearrange("j p f -> p j f"))
            u1csb = cpool.tile([C, BS, EQ], _f32)
            nc.sync.dma_start(out=u1csb[:], in_=u1c_d[:])
            outsb = cpool.tile([C, BS, EQ], _f32)   # [c, (b, w)] staging

            with tc.tile_pool(name="io", bufs=3) as iop, \
                 tc.tile_pool(name="o1", bufs=3) as o1p, \
                 tc.tile_pool(name="scr", bufs=2) as scrp, \
                 tc.tile_pool(name="ps", bufs=2, space="PSUM") as psp:
                for nb in range(BS // NB):
                    bsl = slice(nb * NB, (nb + 1) * NB)
                    t4sb = iop.tile([128, 3, NB, C], _bf, tag="t4")
                    nc.sync.dma_start(out=t4sb[:], in_=trep_d[:, :, bsl])
                    xxsb = iop.tile([C, NB, WCOL], _bf, tag="xx")
                    nc.sync.dma_start(out=xxsb[:], in_=xx_d[:, bsl])

                    n0 = nb * NB
                    ps = psp.tile([128, NB, 512], _f32, tag="ps")
                    for nn in range(NB):
                        for j in range(3):
                            k = KCH[j]
                            nc.tensor.matmul(
                                ps[:, nn, 0:NCOL],
                                t4sb[:k, j, nn, :],
                                u3sb[:k, j, :],
                                start=(j == 0), stop=(j == 2),
                            )

                    o1sb = o1p.tile([128, NB, NCOL], _bf, tag="o1")
                    nc.scalar.copy(o1sb[:], ps[:, :, 0:NCOL])

                    o1v = o1sb[:].rearrange("c n (w f) -> c n w f", w=EQ)
                    xxp = xxsb[:, :, None, :]
                    scr = scrp.tile([128, NB, EQ, WCOL], _bf, tag="scr")
                    nc.vector.tensor_mul(
                        scr[:, :, 0, :],
                        o1v[:, :, 0, :],
                        xxp[:, :, 0, :].to_broadcast([C, NB, WCOL]),
                    )
                    nc.gpsimd.tensor_mul(
                        scr[:, :, 1:3, :],
                        o1v[:, :, 1:3, :],
                        xxp.to_broadcast([C, NB, 2, WCOL]),
                    )
                    nc.vector.tensor_reduce(
                        outsb[:, n0 : n0 + NB, :].rearrange("c n w -> c (n w)"),
                        scr[:].rearrange("c n w f -> c (n w) f"),
                        axis=mybir.AxisListType.X,
                        op=add,
                    )

            # add the U1 terms for all (b, w) in one batched op
            nc.vector.tensor_add(
                outsb[:].rearrange("c b w -> c (b w)"),
                outsb[:].rearrange("c b w -> c (b w)"),
                u1csb[:].rearrange("c b w -> c (b w)"),
            )

            # ---------------- end phase: layout transform ----------------
            with tc.tile_pool(name="fin", bufs=2) as fpool, \
                 tc.tile_pool(name="ps_fin", bufs=2, space="PSUM") as psf:
                ident128 = cpool.tile([128, 128], _f32)
                make_identity(nc, ident128[:])

                finsb = fpool.tile([BS, C * EQ], _f32, tag="finsb")
                finsb_r = finsb[:].rearrange("b (c w) -> b c w", w=EQ)
                for w in range(EQ):
                    fin_ps = psf.tile([BS, C], _f32, tag="fin")
                    nc.tensor.transpose(fin_ps[:], outsb[:, :, w], ident128[:])
                    nc.scalar.copy(finsb_r[:, :, w], fin_ps[:])

                nc.sync.dma_start(out=out_d[:], in_=finsb[:])

    import bass_rust
    bass_rust.move_matmul_waits_to_ldweights(nc.m)
    bass_rust.generate_event_semaphores(nc)
    return nc


def _pair_index():
    """(a, b) pairs with a < b, in fixed enumeration order."""
    pairs = [(a, b) for a in range(ELL) for b in range(a + 1, ELL)]
    assert len(pairs) == NPAIR
    return pairs


def _host_prep(x, y, U3, U2, U1, w_max, w2, w1):
    """Numpy-side input prep. Returns per_core(ci) -> input map."""
    x = np.ascontiguousarray(x, dtype=np.float32)
    elem = np.argmax(y, axis=1)

    wn3 = w_max[elem]                                # [B, 23, C]
    wn2 = w2[elem]                                   # [B, 5, C]
    wn1 = w1[elem][:, 0, :]                          # [B, C]

    # trep[p, j, b, c]: rows r=128j+p<368: x[b,c,r%16]*wn3[b,r//16,c];
    # chunk2 rows 112:117 = wn2; row 117 = wn1; rest 0
    xT = x.transpose(0, 2, 1)                        # [B, 16, C]
    trep = np.zeros((B, 384, C), dtype=np.float32)
    wn3r = np.repeat(wn3, ELL, axis=1)               # [B, 368, C]
    xtile = np.tile(xT, (1, P3, 1))                  # [B, 368, C]
    trep[:, :368] = wn3r * xtile
    trep[:, 368:373] = wn2
    trep[:, 373] = wn1
    trep = trep.reshape(B, 3, 128, C).transpose(2, 1, 0, 3)   # [128, 3, B, C]
    trep = np.ascontiguousarray(trep).astype(bf16)

    pairs = _pair_index()
    pa = np.array([p[0] for p in pairs])
    pb = np.array([p[1] for p in pairs])

    # u3cat [3, 128, 424]: per w-block 136 cols = 120 symmetrized pairs +
    # 16 diagonal; cols 408:424 = u1-ext (w=2 only, row 373 = wn1)
    u3full = np.zeros((384, EQ, ELL, ELL), dtype=np.float32)
    u3full[:368] = U3.transpose(4, 3, 0, 1, 2).reshape(368, EQ, ELL, ELL)
    u3full[368:373] = U2.transpose(3, 0, 1, 2)
    u3cat = np.zeros((384, NCOL), dtype=np.float32)
    for w in range(EQ):
        base = WCOL * w
        u3cat[:, base : base + NPAIR] = (
            u3full[:, w, pa, pb] + u3full[:, w, pb, pa]
        )
        u3cat[:, base + NPAIR : base + WCOL] = u3full[
            :, w, np.arange(ELL), np.arange(ELL)
        ]
    u3cat = u3cat.reshape(3, 128, NCOL).astype(bf16)

    # xx [B, C, 136]: 0:120 x_a*x_b pairs; 120:136 x_u^2
    xxf = np.empty((B, C, WCOL), dtype=np.float32)
    xxf[:, :, :NPAIR] = x[:, :, pa] * x[:, :, pb]
    xxf[:, :, NPAIR:WCOL] = x * x
    xxf = xxf.astype(bf16)

    # u1c [B, C, 3]: wn1 * (U1[w] . x), added host-side style at the end
    u1x = np.einsum("bci,wi->bcw", x, U1[:, :, 0])
    u1c = np.ascontiguousarray(wn1[:, :, None] * u1x)   # [B, C, 3] f32

    def per_core(ci):
        s = slice(ci * BS, (ci + 1) * BS)
        return {
            "trep": np.ascontiguousarray(trep[:, :, s]),
            "xx": np.ascontiguousarray(xxf[s].transpose(1, 0, 2)),
            "u3cat": u3cat,
            "u1c": np.ascontiguousarray(u1c[s].transpose(1, 0, 2)),
        }

    return per_core


_PROGRAM_CACHE = {}


def kernel(**inputs) -> np.ndarray:
    from concourse.bass_utils import run_bass_kernel_spmd

    per_core = _host_prep(
        np.asarray(inputs["x"]), np.asarray(inputs["y"]),
        np.asarray(inputs["U3"]), np.asarray(inputs["U2"]),
        np.asarray(inputs["U1"]), np.asarray(inputs["w_max"]),
        np.asarray(inputs["w2"]), np.asarray(inputs["w1"]),
    )

    if "nc" not in _PROGRAM_CACHE:
        _PROGRAM_CACHE["nc"] = _build_program()
    nc = _PROGRAM_CACHE["nc"]

    in_maps = [per_core(ci) for ci in range(N_CORES)]
    res = run_bass_kernel_spmd(nc, in_maps, core_ids=list(range(N_CORES)))
    out = np.concatenate([r["out"] for r in res.results], axis=0)
    return out.astype(np.float32)


if __name__ == "__main__":
    # smoke test in CoreSim on core 0's shard
    from concourse.bass_interp import CoreSim

    rng = np.random.default_rng(0)
    x = rng.standard_normal((B, C, ELL)).astype(np.float32)
    elem = rng.integers(0, E, size=B)
    y = np.eye(E, dtype=np.float32)[elem]
    U3 = (rng.standard_normal((EQ, ELL, ELL, ELL, P3)) * 0.1).astype(np.float32)
    U2 = (rng.standard_normal((EQ, ELL, ELL, P2)) * 0.1).astype(np.float32)
    U1 = (rng.standard_normal((EQ, ELL, P1)) * 0.1).astype(np.float32)
    w_max = (rng.standard_normal((E, P3, C)) / P3).astype(np.float32)
    w2 = (rng.standard_normal((E, P2, C)) / P2).astype(np.float32)
    w1 = (rng.standard_normal((E, P1, C)) / P1).astype(np.float32)

    per_core = _host_prep(x, y, U3, U2, U1, w_max, w2, w1)
    nc = _build_program()
    sim = CoreSim(nc)
    m = per_core(0)
    for k, v in m.items():
        sim.tensor(k)[:] = v
    sim.simulate(check_with_hw=False, trace_hw=False)
    got = np.array(sim.tensor("out"))
    print(f"sim time: {sim.time} ns")

    def ref_np(x, y, U3, U2, U1, w_max, w2, w1):
        wn3 = np.einsum("be,ekc->bkc", y, w_max)
        t = np.einsum("bkc,bci->bcik", wn3, x)
        out = np.einsum("wxvik,bcik->bcwxv", U3, t)
        wn2 = np.einsum("be,ekc->bkc", y, w2)
        c2 = np.einsum("wxvk,bkc->bcwxv", U2, wn2) + out
        out = np.einsum("bcwxi,bci->bcwx", c2, x)
        wn1 = np.einsum("be,ekc->bkc", y, w1)
        c1 = np.einsum("wxk,bkc->bcwx", U1, wn1) + out
        out = np.einsum("bcwi,bci->bcw", c1, x)
        return out.reshape(out.shape[0], -1)

    want = ref_np(x[:BS], y[:BS], U3, U2, U1, w_max, w2, w1)
    rel = np.linalg.norm(got - want) / (np.linalg.norm(want) + 1e-30)
    err = np.abs(got - want).max() / (np.abs(want).max() + 1e-30)
    print(f"CoreSim vs numpy: l2 rel {rel:.3e}  absmax-rel {err:.3e}")
    assert rel < 2e-2, "FAIL"
    print("SIM PASS")


# revision 10
# speedup vs baseline: 1.2350x; 1.2350x over previous
"""Trainium2 Bass kernel for nn_Contraction (MACE-style CG contraction), v3.

Math (per node b, channel c):
  wn3 = w_max[elem_b] (23,C); wn2 = w2[elem_b] (5,C); wn1 = w1[elem_b] (1,C)
  c2[w,x2,v] = sum_ik U3[w,x2,v,i,k] (x[c,i] wn3[k,c]) + sum_k2 U2[w,x2,v,k2] wn2[k2,c]
  out[c, w]  = sum_{x2,v} c2[w,x2,v] x[c,x2] x[c,v] + sum_x2 U1[w,x2] wn1[c] x[c,x2]

Key reduction: the final sum over (x2, v) is a quadratic form in x, so only
the (x2, v)-symmetric part of c2 matters, and c2 is linear in U3/U2. Host
pre-symmetrizes U3/U2 over (x2, v): 136 columns per w (120 pair-sums + 16
diagonal) instead of 256 — halves the main-matmul moving stream.

Device mapping (per core, BS=128 nodes, one node at a time):
  - transposed main matmul: stationary = t4_n [(ik) 374 rows, c=128]
    (host-built: x*wn3 rows; chunk2 rows 112:117 = wn2 for the folded U2
    term, row 117 = wn1 for the folded U1 term), moving = u3cat
    [(ik), 408] (bf16, const in SBUF). Output out1T[c, 408] f32, ONE PSUM
    bank. Columns: [w0 136 | w1 136 | w2 136].
    3 matmuls/node (one per K chunk 128/128/118).
  - nodes are processed in batches of 4 to amortize per-op fixed
    costs: PSUM tile [128, 4, 512] (4 banks x 2 bufs = all 8 banks; the
    end-phase pool opens after this one closes), ACT convert-copies all
    4 nodes' 408 cols to SBUF bf16 in one op.
  - final contraction: multiply o1 by xx_n[c, 136] broadcast over w
    (DVE: w0+half w1; GPSIMD: rest), then one DVE windowed
    tensor_reduce(axis=X) [c,12,136]->[c,12] writes straight into
    outsb[c, (b, w)] f32. The U1 terms (host-computed u1c) are added
    once at the end in a single batched add.
  - end phase: 3 PE transposes [C,BS] -> [BS,C] into (b, c, w) layout,
    single contiguous DMA out.

Sharding: data-parallel over nodes b across 8 cores (128 nodes/core).
Host prep (numpy): elem gather, t4/u3cat/xx/u1c packing, bf16.
"""

import sys

if "/opt/trn_rl_repo" not in sys.path:
    sys.path.insert(0, "/opt/trn_rl_repo")

import numpy as np
import ml_dtypes

import concourse.bass as bass
import concourse.mybir as mybir
import concourse.tile as tile
from concourse.masks import make_identity

dt = mybir.dt
bf16 = ml_dtypes.bfloat16

# problem constants (hardcoded per contract)
B, C, ELL, EQ, E = 1024, 128, 16, 3, 10
P3, P2, P1 = 23, 5, 1
N_CORES = 8
BS = B // N_CORES          # nodes per core
NB = 4                     # nodes per DMA batch (2 pairs)
NPAIR = ELL * (ELL - 1) // 2      # 120 off-diagonal (x2<v) pairs
WCOL = NPAIR + ELL                # 136 columns per w
NCOL = EQ * WCOL                  # 408 total out1 columns
KCH = (128, 128, 118)      # K chunks (chunk2: 112 U3 + 5 U2 + 1 U1 rows)

_f32 = dt.float32
_bf = dt.bfloat16


def _build_program():
    nc = bass.Bass("TRN2", target_bir_lowering=False, debug=False)

    trep_d = nc.dram_tensor("trep", [128, BS, 3, C], _bf, kind="ExternalInput")
    xx_d = nc.dram_tensor("xx", [C, BS, WCOL], _bf, kind="ExternalInput")
    u3_d = nc.dram_tensor("u3cat", [3, 128, NCOL], _bf, kind="ExternalInput")
    u1c_d = nc.dram_tensor("u1c", [C, BS, EQ], _f32, kind="ExternalInput")
    out_d = nc.dram_tensor("out", [BS, C * EQ], _f32, kind="ExternalOutput")

    mult = mybir.AluOpType.mult
    add = mybir.AluOpType.add

    with tile.TileContext(nc) as tc:
        with tc.tile_pool(name="const", bufs=1) as cpool:
            u3sb = cpool.tile([128, 3, NCOL], _bf)
            nc.sync.dma_start(out=u3sb[:], in_=u3_d[:].rearrange("j p f -> p j f"))
            u1csb = cpool.tile([C, BS, EQ], _f32)
            nc.sync.dma_start(out=u1csb[:], in_=u1c_d[:])
            outsb = cpool.tile([C, BS, EQ], _f32)   # [c, (b, w)] staging

            with tc.tile_pool(name="io", bufs=3) as iop, \
                 tc.tile_pool(name="o1", bufs=3) as o1p, \
                 tc.tile_pool(name="scr", bufs=2) as scrp, \
                 tc.tile_pool(name="ps", bufs=2, space="PSUM") as psp:
                for nb in range(BS // NB):
                    bsl = slice(nb * NB, (nb + 1) * NB)
                    t4sb = iop.tile([128, NB, 3, C], _bf, tag="t4")
                    nc.sync.dma_start(out=t4sb[:], in_=trep_d[:, bsl])
                    xxsb = iop.tile([C, NB, WCOL], _bf, tag="xx")
                    nc.sync.dma_start(out=xxsb[:], in_=xx_d[:, bsl])

                    n0 = nb * NB
                    ps = psp.tile([128, NB, 512], _f32, tag="ps")
                    for nn in range(NB):
                        for j in range(3):
                            k = KCH[j]
                            nc.tensor.matmul(
                                ps[:, nn, 0:NCOL],
                                t4sb[:k, nn, j, :],
                                u3sb[:k, j, :],
                                start=(j == 0), stop=(j == 2),
                            )

                    o1sb = o1p.tile([128, NB, NCOL], _bf, tag="o1")
                    nc.scalar.copy(o1sb[:], ps[:, :, 0:NCOL])

                    o1v = o1sb[:].rearrange("c n (w f) -> c n w f", w=EQ)
                    xxp = xxsb[:, :, None, :]
                    scr = scrp.tile([128, NB, EQ, WCOL], _bf, tag="scr")
                    nc.vector.tensor_mul(
                        scr[:, :, 0, :],
                        o1v[:, :, 0, :],
                        xxp[:, :, 0, :].to_broadcast([C, NB, WCOL]),
                    )
                    nc.vector.tensor_mul(
                        scr[:, :, 1, 0:68],
                        o1v[:, :, 1, 0:68],
                        xxp[:, :, 0, 0:68].to_broadcast([C, NB, 68]),
                    )
                    nc.gpsimd.tensor_mul(
                        scr[:, :, 1, 68:WCOL],
                        o1v[:, :, 1, 68:WCOL],
                        xxp[:, :, 0, 68:WCOL].to_broadcast([C, NB, WCOL - 68]),
                    )
                    nc.gpsimd.tensor_mul(
                        scr[:, :, 2, :],
                        o1v[:, :, 2, :],
                        xxp[:, :, 0, :].to_broadcast([C, NB, WCOL]),
                    )
                    nc.vector.tensor_reduce(
                        outsb[:, n0 : n0 + NB, :].rearrange("c n w -> c (n w)"),
                        scr[:].rearrange("c n w f -> c (n w) f"),
                        axis=mybir.AxisListType.X,
                        op=add,
                    )

            # add the U1 terms for all (b, w) in one batched op
            nc.vector.tensor_add(
                outsb[:].rearrange("c b w -> c (b w)"),
                outsb[:].rearrange("c b w -> c (b w)"),
                u1csb[:].rearrange("c b w -> c (b w)"),
            )

            # ---------------- end phase: layout transform ----------------
            with tc.tile_pool(name="fin", bufs=2) as fpool, \
                 tc.tile_pool(name="ps_fin", bufs=2, space="PSUM") as psf:
                ident128 = cpool.tile([128, 128], _f32)
                make_identity(nc, ident128[:])

                finsb = fpool.tile([BS, C * EQ], _f32, tag="finsb")
                finsb_r = finsb[:].rearrange("b (c w) -> b c w", w=EQ)
                for w in range(EQ):
                    fin_ps = psf.tile([BS, C], _f32, tag="fin")
                    nc.tensor.transpose(fin_ps[:], outsb[:, :, w], ident128[:])
                    nc.scalar.copy(finsb_r[:, :, w], fin_ps[:])

                nc.sync.dma_start(out=out_d[:], in_=finsb[:])

    import bass_rust
    bass_rust.move_matmul_waits_to_ldweights(nc.m)
    bass_rust.generate_event_semaphores(nc)
    return nc


def _pair_index():
    """(a, b) pairs with a < b, in fixed enumeration order."""
    pairs = [(a, b) for a in range(ELL) for b in range(a + 1, ELL)]
    assert len(pairs) == NPAIR
    return pairs


def _host_prep(x, y, U3, U2, U1, w_max, w2, w1):
    """Numpy-side input prep. Returns per_core(ci) -> input map."""
    x = np.ascontiguousarray(x, dtype=np.float32)
    elem = np.argmax(y, axis=1)

    wn3 = w_max[elem]                                # [B, 23, C]
    wn2 = w2[elem]                                   # [B, 5, C]
    wn1 = w1[elem][:, 0, :]                          # [B, C]

    # trep[p, j, b, c]: rows r=128j+p<368: x[b,c,r%16]*wn3[b,r//16,c];
    # chunk2 rows 112:117 = wn2; row 117 = wn1; rest 0
    xT = x.transpose(0, 2, 1)                        # [B, 16, C]
    trep = np.zeros((B, 384, C), dtype=np.float32)
    wn3r = np.repeat(wn3, ELL, axis=1)               # [B, 368, C]
    xtile = np.tile(xT, (1, P3, 1))                  # [B, 368, C]
    trep[:, :368] = wn3r * xtile
    trep[:, 368:373] = wn2
    trep[:, 373] = wn1
    trep = trep.reshape(B, 3, 128, C).transpose(2, 0, 1, 3)   # [128, B, 3, C]
    trep = np.ascontiguousarray(trep).astype(bf16)

    pairs = _pair_index()
    pa = np.array([p[0] for p in pairs])
    pb = np.array([p[1] for p in pairs])

    # u3cat [3, 128, 424]: per w-block 136 cols = 120 symmetrized pairs +
    # 16 diagonal; cols 408:424 = u1-ext (w=2 only, row 373 = wn1)
    u3full = np.zeros((384, EQ, ELL, ELL), dtype=np.float32)
    u3full[:368] = U3.transpose(4, 3, 0, 1, 2).reshape(368, EQ, ELL, ELL)
    u3full[368:373] = U2.transpose(3, 0, 1, 2)
    u3cat = np.zeros((384, NCOL), dtype=np.float32)
    for w in range(EQ):
        base = WCOL * w
        u3cat[:, base : base + NPAIR] = (
            u3full[:, w, pa, pb] + u3full[:, w, pb, pa]
        )
        u3cat[:, base + NPAIR : base + WCOL] = u3full[
            :, w, np.arange(ELL), np.arange(ELL)
        ]
    u3cat = u3cat.reshape(3, 128, NCOL).astype(bf16)

    # xx [B, C, 136]: 0:120 x_a*x_b pairs; 120:136 x_u^2
    xxf = np.empty((B, C, WCOL), dtype=np.float32)
    xxf[:, :, :NPAIR] = x[:, :, pa] * x[:, :, pb]
    xxf[:, :, NPAIR:WCOL] = x * x
    xxf = xxf.astype(bf16)

    # u1c [B, C, 3]: wn1 * (U1[w] . x), added host-side style at the end
    u1x = np.einsum("bci,wi->bcw", x, U1[:, :, 0])
    u1c = np.ascontiguousarray(wn1[:, :, None] * u1x)   # [B, C, 3] f32

    def per_core(ci):
        s = slice(ci * BS, (ci + 1) * BS)
        return {
            "trep": np.ascontiguousarray(trep[:, s]),
            "xx": np.ascontiguousarray(xxf[s].transpose(1, 0, 2)),
            "u3cat": u3cat,
            "u1c": np.ascontiguousarray(u1c[s].transpose(1, 0, 2)),
        }

    return per_core


_PROGRAM_CACHE = {}


def kernel(**inputs) -> np.ndarray:
    from concourse.bass_utils import run_bass_kernel_spmd

    per_core = _host_prep(
        np.asarray(inputs["x"]), np.asarray(inputs["y"]),
        np.asarray(inputs["U3"]), np.asarray(inputs["U2"]),
        np.asarray(inputs["U1"]), np.asarray(inputs["w_max"]),
        np.asarray(inputs["w2"]), np.asarray(inputs["w1"]),
    )

    if "nc" not in _PROGRAM_CACHE:
        _PROGRAM_CACHE["nc"] = _build_program()
    nc = _PROGRAM_CACHE["nc"]

    in_maps = [per_core(ci) for ci in range(N_CORES)]
    res = run_bass_kernel_spmd(nc, in_maps, core_ids=list(range(N_CORES)))
    out = np.concatenate([r["out"] for r in res.results], axis=0)
    return out.astype(np.float32)


if __name__ == "__main__":
    # smoke test in CoreSim on core 0's shard
    from concourse.bass_interp import CoreSim

    rng = np.random.default_rng(0)
    x = rng.standard_normal((B, C, ELL)).astype(np.float32)
    elem = rng.integers(0, E, size=B)
    y = np.eye(E, dtype=np.float32)[elem]
    U3 = (rng.standard_normal((EQ, ELL, ELL, ELL, P3)) * 0.1).astype(np.float32)
    U2 = (rng.standard_normal((EQ, ELL, ELL, P2)) * 0.1).astype(np.float32)
    U1 = (rng.standard_normal((EQ, ELL, P1)) * 0.1).astype(np.float32)
    w_max = (rng.standard_normal((E, P3, C)) / P3).astype(np.float32)
    w2 = (rng.standard_normal((E, P2, C)) / P2).astype(np.float32)
    w1 = (rng.standard_normal((E, P1, C)) / P1).astype(np.float32)

    per_core = _host_prep(x, y, U3, U2, U1, w_max, w2, w1)
    nc = _build_program()
    sim = CoreSim(nc)
    m = per_core(0)
    for k, v in m.items():
        sim.tensor(k)[:] = v
    sim.simulate(check_with_hw=False, trace_hw=False)
    got = np.array(sim.tensor("out"))
    print(f"sim time: {sim.time} ns")

    def ref_np(x, y, U3, U2, U1, w_max, w2, w1):
        wn3 = np.einsum("be,ekc->bkc", y, w_max)
        t = np.einsum("bkc,bci->bcik", wn3, x)
        out = np.einsum("wxvik,bcik->bcwxv", U3, t)
        wn2 = np.einsum("be,ekc->bkc", y, w2)
        c2 = np.einsum("wxvk,bkc->bcwxv", U2, wn2) + out
        out = np.einsum("bcwxi,bci->bcwx", c2, x)
        wn1 = np.einsum("be,ekc->bkc", y, w1)
        c1 = np.einsum("wxk,bkc->bcwx", U1, wn1) + out
        out = np.einsum("bcwi,bci->bcw", c1, x)
        return out.reshape(out.shape[0], -1)

    want = ref_np(x[:BS], y[:BS], U3, U2, U1, w_max, w2, w1)
    rel = np.linalg.norm(got - want) / (np.linalg.norm(want) + 1e-30)
    err = np.abs(got - want).max() / (np.abs(want).max() + 1e-30)
    print(f"CoreSim vs numpy: l2 rel {rel:.3e}  absmax-rel {err:.3e}")
    assert rel < 2e-2, "FAIL"
    print("SIM PASS")
